# revision 1
# baseline (speedup 1.0000x reference)
"""GNN decoder kernel for Trainium2 (8 NeuronCores, SPMD data-parallel over graphs).

Computation (see reference):
    offsets[g] = first global node index of graph g (from sorted batch_ids)
    gi[g,e]    = clip(offsets[g] + targets[g,e], 0, N-1)
    q[g]       = concat(emb[gi[g,0]], emb[gi[g,1]])          # [B, 512]
    out        = q @ W + b                                    # [B, 128]

Device work per core (512 graphs): 8 indirect-DMA gathers bring in the 1024
query embedding rows (one index per partition per op — the multi-index form
returns garbage on real HW), 16 PE transposes put the feature dim on
partitions, 16 fp32 matmuls accumulate q @ W in PSUM, DVE adds the bias,
one DMA stores. Host does index math + sharding only.

Raw (non-Tile) engine programs with explicit semaphores: Tile's entry
event-semaphore chains and tail drain/EVSEM butterfly cost ~12us on a ~20us
kernel, and TRN2 allows only one sync wait per instruction (raw standalone
wait_ge sidesteps that). Gathers are issued pairwise per graph-chunk so the
PE pipeline chases the gather stream instead of waiting for all of it.

PSUM bank discipline (PE-write + DVE-read of one bank is a HW-fatal race):
each transpose group (ptq, [128,512] = 1 full bank per graph-chunk) is read
by DVE only after its 4th transpose; each matmul accumulator (po, own bank)
is read by DVE only after its 4th matmul; PE never revisits a bank.
"""

import numpy as np

import concourse.bass as bass
import concourse.bacc as bacc
import concourse.mybir as mybir
from concourse.bass_utils import run_bass_kernel_spmd

N_NODES = 262144
N_GRAPHS = 4096
D = 256            # embedding dim
TS = 128           # target size (output features)
N_CORES = 8
GPC = N_GRAPHS // N_CORES   # 512 graphs per core
F32 = mybir.dt.float32
I32 = mybir.dt.int32

# constants-tensor column layout (f32 [128, 768])
C_W = 0            # [128, 512]  w[f, fc*128+o] = W[fc*128+f, o]
C_B = 512          # [128, 128]  bias replicated over partitions
C_ID = 640         # [128, 128]  identity for PE transpose
C_COLS = 768

# cleared in sim runs: CoreSim's race detector rejects sem_clear-after-drain
# (conservative), while HW needs the teardown for clean NEFF re-execution
TEARDOWN = True


def build_program() -> bass.Bass:
    nc = bacc.Bacc("TRN2", target_bir_lowering=False, debug=False)

    emb = nc.dram_tensor("emb", [N_NODES, D], F32, kind="ExternalInput")
    idx = nc.dram_tensor("idx", [128, 8], I32, kind="ExternalInput")
    cin = nc.dram_tensor("cin", [128, C_COLS], F32, kind="ExternalInput")
    out = nc.dram_tensor("out", [GPC, TS], F32, kind="ExternalOutput")

    idx_sb = nc.alloc_sbuf_tensor("idx_sb", [128, 8], I32)
    cin_sb = nc.alloc_sbuf_tensor("cin_sb", [128, C_COLS], F32)
    g_sb = [nc.alloc_sbuf_tensor(f"g{t}", [128, D], F32) for t in range(8)]
    qt_sb = [nc.alloc_sbuf_tensor(f"qt{gc}", [128, 512], F32) for gc in range(4)]
    out_sb = nc.alloc_sbuf_tensor("o_sb", [128, 4 * TS], F32)

    ptq = [nc.alloc_psum_tensor(f"ptq{gc}", [128, 512], F32) for gc in range(4)]
    po = [nc.alloc_psum_tensor(f"po{gc}", [128, TS], F32) for gc in range(4)]

    s_idx = nc.alloc_semaphore("s_idx")
    s_cin = nc.alloc_semaphore("s_cin")
    s_g = [[nc.alloc_semaphore(f"s_g{e}_{gc}") for gc in range(4)] for e in range(2)]
    s_pe = nc.alloc_semaphore("s_pe")
    s_qt = nc.alloc_semaphore("s_qt")
    s_mm = nc.alloc_semaphore("s_mm")
    s_add = nc.alloc_semaphore("s_add")
    s_out = nc.alloc_semaphore("s_out")

    w_t = cin_sb[:, C_W : C_W + 512]
    b_t = cin_sb[:, C_B : C_B + TS]
    ident = cin_sb[:, C_ID : C_ID + 128]

    with nc.Block() as block:

        @block.sync
        def _(sync):
            sync.dma_start(out=idx_sb[:], in_=idx[:, :]).then_inc(s_idx, 16)
            sync.dma_start(out=cin_sb[:], in_=cin[:, :]).then_inc(s_cin, 16)
            sync.wait_ge(s_add, 4)
            sync.dma_start(
                out=out[:, :].rearrange("(gc p) o -> p gc o", p=128),
                in_=out_sb[:].rearrange("p (gc o) -> p gc o", gc=4),
            ).then_inc(s_out, 16)

        @block.gpsimd
        def _(gpsimd):
            gpsimd.wait_ge(s_idx, 16)
            # pairwise per graph-chunk: (e0,gc), (e1,gc) so PE can finish
            # chunk gc while later chunks still gather
            for gc in range(4):
                for e in range(2):
                    t = e * 4 + gc
                    gpsimd.indirect_dma_start(
                        out=g_sb[t][:],
                        out_offset=None,
                        in_=emb[:, :],
                        in_offset=bass.IndirectOffsetOnAxis(
                            ap=idx_sb[:, t : t + 1], axis=0
                        ),
                    ).then_inc(s_g[e][gc], 16)
            # teardown: zero all semaphores once everything (incl. the output
            # store) completed, so re-executing the loaded NEFF starts clean
            gpsimd.wait_ge(s_out, 16)
            if TEARDOWN:
                gpsimd.dma_reset(range(s_idx.num, s_out.num + 1))
                gpsimd.sem_clear(range(s_idx.num, s_out.num + 1))

        @block.tensor
        def _(tensor):
            tensor.wait_ge(s_cin, 16)

            def t_half(gc, e, inc=False):
                tensor.wait_ge(s_g[e][gc], 16)
                for c in range(2):
                    fc = 2 * e + c
                    ins = nc.tensor.transpose(
                        out=ptq[gc][:, fc * 128 : (fc + 1) * 128],
                        in_=g_sb[e * 4 + gc][:, c * 128 : (c + 1) * 128],
                        identity=ident,
                    )
                if inc:
                    ins.then_inc(s_pe, 1)

            def t_group(gc):
                t_half(gc, 0)
                t_half(gc, 1, inc=True)

            def m_group(gc):
                tensor.wait_ge(s_qt, gc + 1)
                for fc in range(4):
                    ins = nc.tensor.matmul(
                        out=po[gc][:, 0:TS],
                        lhsT=qt_sb[gc][:, fc * 128 : (fc + 1) * 128],
                        rhs=w_t[:, fc * 128 : (fc + 1) * 128],
                        start=(fc == 0),
                        stop=(fc == 3),
                    )
                ins.then_inc(s_mm, 1)

            t_group(0)
            t_group(1)
            m_group(0)
            t_group(2)
            m_group(1)
            # weave the last chunk so only 2 transposes + 4 matmuls remain
            # after the final gather lands
            t_half(3, 0)
            m_group(2)
            t_half(3, 1, inc=True)
            m_group(3)

        @block.vector
        def _(vector):
            vector.wait_ge(s_cin, 16)

            def c_group(gc):
                vector.wait_ge(s_pe, gc + 1)
                nc.vector.tensor_copy(out=qt_sb[gc][:], in_=ptq[gc][:]).then_inc(
                    s_qt, 1
                )

            def a_group(gc):
                vector.wait_ge(s_mm, gc + 1)
                nc.vector.tensor_add(
                    out=out_sb[:, gc * TS : (gc + 1) * TS],
                    in0=po[gc][:, 0:TS],
                    in1=b_t,
                ).then_inc(s_add, 1)

            c_group(0)
            c_group(1)
            a_group(0)
            c_group(2)
            a_group(1)
            c_group(3)
            a_group(2)
            a_group(3)

    nc.compile()
    return nc


_PROG = None


def _get_prog() -> bass.Bass:
    global _PROG
    if _PROG is None:
        _PROG = build_program()
    return _PROG


def make_in_maps(batch_emb, batch_ids, targets, W, b):
    emb = np.ascontiguousarray(np.asarray(batch_emb, dtype=np.float32))
    ids = np.asarray(batch_ids)
    tg = np.asarray(targets)

    # offsets[g] = exclusive prefix count = first index of graph g in sorted ids
    offsets = np.searchsorted(ids, np.arange(N_GRAPHS, dtype=np.int64), side="left")
    gi = offsets[:, None] + tg.astype(np.int64)
    gi = np.clip(gi, 0, N_NODES - 1).astype(np.int32)  # match jax clamp semantics

    w_re = np.asarray(W, dtype=np.float32).reshape(4, 128, TS).transpose(1, 0, 2).reshape(128, 4 * TS)
    b_rep = np.broadcast_to(np.asarray(b, dtype=np.float32), (128, TS))
    ident = np.eye(128, dtype=np.float32)
    cin = np.ascontiguousarray(np.concatenate([w_re, b_rep, ident], axis=1))

    in_maps = []
    for k in range(N_CORES):
        blk = gi[k * GPC : (k + 1) * GPC]  # [512, 2]
        idx_k = np.empty((128, 8), np.int32)
        for e in range(2):
            for gc in range(4):
                idx_k[:, e * 4 + gc] = blk[gc * 128 : (gc + 1) * 128, e]
        in_maps.append({"emb": emb, "idx": idx_k, "cin": cin})
    return in_maps


def kernel(batch_emb, batch_ids, targets, W, b):
    in_maps = make_in_maps(batch_emb, batch_ids, targets, W, b)
    res = run_bass_kernel_spmd(_get_prog(), in_maps, list(range(N_CORES)))
    return np.concatenate([res.results[k]["out"] for k in range(N_CORES)], axis=0)

